# revision 14
# baseline (speedup 1.0000x reference)
"""Trainium2 Bass kernel for nn_ACRoPEAttention (axial RoPE attention).

Sharding: sequence-parallel. 8 cores = 2 batches x 4 token-chunks of 512.
Each core computes q/k/v (all 16 heads) for its 512 tokens, RoPEs them,
AllGathers k^T and v within its 4-core batch group, runs full attention for
its 512 queries, and projects. Output is token-sharded -> host concat.

Device dataflow is fully "transposed" (channels on partitions):
  qkv^T = Wqkv^T-as-lhsT matmuls over x^T;  RoPE pair-swap via a constant
  S-matrix matmul;  scores^T per head = k^T-as-lhsT @ q^T (row-paired 2
  heads/matmul, explicit ldweights hoisting);  softmax = exp (no max-sub;
  scores are O(1)); denominator + att@v as a uniform (128,32)-tile col grid
  on the PE (4x M=32 att@v tiles + 2x M=32 ones-tiles);  y^T =
  Wproj^T-as-lhsT @ attn^T.  All matmul operands bf16 (fp32 accum).

Attention is a flat 128-slot pipeline (8 pairs x 16 key-groups): scores+exp
stream at ACT rate; the denominator matmuls trail by LAG_D slots; the att@v
matmuls trail by LAG_AV slots so the in-order PE queue never blocks on the
AllGather-v readback.
"""

import sys

import numpy as np
import ml_dtypes

if "/opt/trn_rl_repo" not in sys.path:
    sys.path.insert(0, "/opt/trn_rl_repo")

BF16 = ml_dtypes.bfloat16

NUM_HEADS = 16
GRID_SIZE = 16
B, N, C = 2, 2048, 1024
HD = C // NUM_HEADS          # 64
NCORES = 8
CHUNK = N // 4               # 512 tokens per core
NPAIR = NUM_HEADS // 2       # 8 head-pair tiles of 128 partitions
KCH = N // 128               # 16 k-chunks of 128 tokens
NSLOT = NPAIR * KCH          # 128 global (pair, group) slots

_CACHE = {}


# ----------------------------------------------------------------- host prep

def _build_tables(T, H, W):
    """Full-token cos/sin tables [N, 64] float64 (tiled-repeat RoPE layout)."""
    n = T * H * W
    ids = np.arange(n)
    d_pos = (ids // (H * W)).astype(np.float64)
    rem = ids % (H * W)
    h_pos = (rem // W).astype(np.float64) * (GRID_SIZE / H)
    w_pos = (rem % W).astype(np.float64) * (GRID_SIZE / W)
    half = 10
    omega = 1.0 / (10000.0 ** (np.arange(half, dtype=np.float64) / half))
    cos_full = np.ones((n, HD), np.float64)
    sin_full = np.zeros((n, HD), np.float64)
    for seg, pos in enumerate([d_pos, h_pos, w_pos]):
        freq = pos[:, None] * omega[None, :]
        cos_full[:, seg * 20:(seg + 1) * 20] = np.tile(np.cos(freq), (1, 2))
        sin_full[:, seg * 20:(seg + 1) * 20] = np.tile(np.sin(freq), (1, 2))
    return cos_full, sin_full


def _build_S128():
    """S such that matmul(out, lhsT=S, rhs=q^T) gives out[2i]=-q[2i+1],
    out[2i+1]=q[2i] for dims<60 of each 64-dim head block (2 blocks)."""
    S = np.zeros((128, 128), np.float32)
    for blk in (0, 64):
        for i in range(30):
            S[blk + 2 * i + 1, blk + 2 * i] = -1.0
            S[blk + 2 * i, blk + 2 * i + 1] = 1.0
    return S


# ------------------------------------------------------------- graph builder

def _build_nc(reps=1, fake_ag=False, probe=None, lag_av=36, lag_d=2):
    import concourse.bass as bass
    import concourse.mybir as mybir
    import concourse.tile as tile
    from concourse import bacc

    f32 = mybir.dt.float32
    bf16 = mybir.dt.bfloat16
    Exp = mybir.ActivationFunctionType.Exp
    mult = mybir.AluOpType.mult
    add = mybir.AluOpType.add

    nc = bacc.Bacc(None, num_devices=NCORES)

    # parameters (per-core shards / replicated)
    xT = nc.declare_dram_parameter("xT", [C, CHUNK], bf16, isOutput=False)
    wqkvT = nc.declare_dram_parameter("wqkvT", [C, 3 * C], bf16, isOutput=False)
    wprojT = nc.declare_dram_parameter("wprojT", [C, C], bf16, isOutput=False)
    bproj = nc.declare_dram_parameter("bproj", [C, 1], f32, isOutput=False)
    cosq_p = nc.declare_dram_parameter("cosq", [128, CHUNK], f32, isOutput=False)
    sinq_p = nc.declare_dram_parameter("sinq", [128, CHUNK], f32, isOutput=False)
    cosk_p = nc.declare_dram_parameter("cosk", [128, CHUNK], f32, isOutput=False)
    sink_p = nc.declare_dram_parameter("sink", [128, CHUNK], f32, isOutput=False)
    smat_p = nc.declare_dram_parameter("smat", [128, 128], bf16, isOutput=False)
    selb_p = nc.declare_dram_parameter("selb", [4, 256], mybir.dt.float32r,
                                       isOutput=False)
    gath_p = nc.declare_dram_parameter("gath", [128, 4], mybir.dt.float32r,
                                       isOutput=False)
    out_p = nc.declare_dram_parameter("out", [C, CHUNK], f32, isOutput=True)

    with tile.TileContext(nc) as tc:
        with (
            tc.tile_pool(name="const", bufs=1) as constp,
            tc.tile_pool(name="dram", bufs=1, space="DRAM") as dramp,
            tc.tile_pool(name="qro", bufs=8) as qrop,
            tc.tile_pool(name="kf", bufs=8) as kfp,
            tc.tile_pool(name="vf", bufs=16) as vfp,
            tc.tile_pool(name="wp", bufs=8) as wpp,
            tc.tile_pool(name="attn", bufs=8) as attnp,
            tc.tile_pool(name="avsb", bufs=3) as avsbp,
            tc.tile_pool(name="dens", bufs=1) as densp,
        ):
            # ---- constants
            cosq = constp.tile([128, CHUNK], f32, tag="cosq")
            sinq = constp.tile([128, CHUNK], f32, tag="sinq")
            cosk = constp.tile([128, CHUNK], f32, tag="cosk")
            sink = constp.tile([128, CHUNK], f32, tag="sink")
            smat = constp.tile([128, 128], bf16, tag="smat")
            ones32 = constp.tile([128, 32], bf16, tag="ones32")
            selb = constp.tile([4, 256], mybir.dt.float32r, tag="selb")
            gath = constp.tile([128, 4], mybir.dt.float32r, tag="gath")
            bias = constp.tile([128, 8], f32, tag="bias")
            nc.sync.dma_start(selb, selb_p[:, :])
            nc.sync.dma_start(gath, gath_p[:, :])
            nc.sync.dma_start(cosq, cosq_p[:, :])
            nc.sync.dma_start(sinq, sinq_p[:, :])
            nc.sync.dma_start(cosk, cosk_p[:, :])
            nc.sync.dma_start(sink, sink_p[:, :])
            nc.sync.dma_start(smat, smat_p[:, :])
            nc.vector.memset(ones32, 1.0)
            # bias [1024,1] -> [128, 8]: col c = b[c*128 : (c+1)*128]
            for c in range(8):
                nc.sync.dma_start(bias[:, c:c + 1],
                                  bproj[c * 128:(c + 1) * 128, :])

            for rep in range(reps):
                # ---- bounce buffers: AGk split in two halves + AGv, ordered
                # AGk1 -> AGv -> AGk2 so the scores stream unblocks earliest
                # and v arrives before the trailing att@v matmuls need it.
                agink1 = dramp.tile([4, 128, 512], bf16, tag="agink1")
                agoutk1 = dramp.tile([4, 4, 128, 512], bf16, tag="agoutk1")
                agink2 = dramp.tile([4, 128, 512], bf16, tag="agink2")
                agoutk2 = dramp.tile([4, 4, 128, 512], bf16, tag="agoutk2")
                aginv = dramp.tile([8, 128, 512], bf16, tag="aginv")
                agoutv = dramp.tile([4, 8, 128, 512], bf16, tag="agoutv")

                with (
                    tc.tile_pool(name="xw", bufs=1) as xwp,
                    tc.tile_pool(name="kvloc", bufs=1) as kvlocp,
                    tc.tile_pool(name="rtmp", bufs=3) as rtmpp,
                    tc.tile_pool(name="qkpsum", bufs=6, space="PSUM") as qkpsump,
                    tc.tile_pool(name="swpsum", bufs=2, space="PSUM") as swpsump,
                ):
                    # ---- load x^T and W_qkv^T, pairwise-interleaved
                    xt = []
                    wq = []
                    for c in range(8):
                        t = xwp.tile([128, CHUNK], bf16, name=f"xt{c}", tag=f"xt{c}")
                        xt.append(t)
                        w = xwp.tile([128, 3 * C], bf16, name=f"wq{c}", tag=f"wq{c}")
                        wq.append(w)
                        nc.sync.dma_start(t, xT[c * 128:(c + 1) * 128, :])
                        nc.sync.dma_start(w[:, C:2 * C],
                                          wqkvT[c * 128:(c + 1) * 128, C:2 * C])
                    for lo, hi in ((0, C), (2 * C, 3 * C)):
                        for c in range(8):
                            nc.sync.dma_start(
                                wq[c][:, lo:hi],
                                wqkvT[c * 128:(c + 1) * 128, lo:hi])

                    def rope_stage1(p, which):
                        """qkv matmul group + PSUM->SBUF cast for pair p."""
                        off = (0 if which == "q" else C) + p * 128
                        ps = qkpsump.tile([128, CHUNK], f32, name=f"{which}ps{p}",
                                          tag="qkps")
                        for c in range(8):
                            nc.tensor.matmul(ps, wq[c][:, off:off + 128], xt[c],
                                             start=(c == 0), stop=(c == 7))
                        raw = rtmpp.tile([128, CHUNK], bf16, name=f"{which}raw{p}",
                                         tag="raw")
                        nc.vector.tensor_copy(raw, ps)
                        return raw

                    def rope_stage2(p, which, raw):
                        """S-matmul swap + cos/sin combine (one pair late)."""
                        cost = cosq if which == "q" else cosk
                        sint = sinq if which == "q" else sink
                        sw = swpsump.tile([128, CHUNK], f32, name=f"{which}sw{p}",
                                          tag="swp")
                        nc.tensor.matmul(sw, smat, raw, start=True, stop=True)
                        t1 = rtmpp.tile([128, CHUNK], f32, name=f"{which}t1{p}",
                                        tag="t1")
                        nc.vector.tensor_tensor(t1, sw, sint, mult)
                        t2 = rtmpp.tile([128, CHUNK], f32, name=f"{which}t2{p}",
                                        tag="t2")
                        nc.vector.tensor_tensor(t2, raw, cost, mult)
                        if which == "q":
                            ro = qrop.tile([128, CHUNK], bf16, name=f"qro{p}", tag="qro")
                        else:
                            ro = kvlocp.tile([128, CHUNK], bf16, name=f"kro{p}",
                                             tag=f"kro{p}")
                        nc.vector.tensor_tensor(ro, t1, t2, add)
                        return ro

                    # ---- k pairs 0-3 -> AGk1; then v -> AGv; then k pairs
                    # 4-7 -> AGk2; q last.
                    kraws = {}
                    for p in range(5):
                        if p < 4:
                            kraws[p] = rope_stage1(p, "k")
                        if p >= 1:
                            kro = rope_stage2(p - 1, "k", kraws.pop(p - 1))
                            nc.sync.dma_start(agink1[p - 1], kro)

                    if fake_ag:
                        for j in range(4):
                            nc.sync.dma_start(agoutk1[j], agink1)
                    else:
                        nc.gpsimd.collective_compute(
                            "AllGather",
                            mybir.AluOpType.bypass,
                            replica_groups=[[0, 1, 2, 3], [4, 5, 6, 7]],
                            ins=[agink1.opt()],
                            outs=[agoutk1.opt()],
                        )

                    # k^T readback for pairs 0-3 immediately after AGk1
                    kf = [None] * NPAIR
                    for p in range(4):
                        t = kfp.tile([128, N], bf16, name=f"kf{p}", tag="kf")
                        nc.sync.dma_start(
                            t, agoutk1[:, p, :, :].rearrange("r p e -> p r e"))
                        kf[p] = t

                    for i in range(4):
                        vloc = kvlocp.tile([128, C], bf16, name=f"vloc{i}",
                                           tag=f"vloc{i}")
                        for oc in range(2):
                            ps = qkpsump.tile([128, 512], f32, name=f"vps{i}{oc}",
                                              tag="qkps")
                            for c in range(8):
                                nc.tensor.matmul(
                                    ps,
                                    xt[c][:, i * 128:(i + 1) * 128],
                                    wq[c][:, 2 * C + oc * 512:2 * C + (oc + 1) * 512],
                                    start=(c == 0), stop=(c == 7))
                            nc.vector.tensor_copy(vloc[:, oc * 512:(oc + 1) * 512], ps)
                        nc.sync.dma_start(aginv[2 * i], vloc[:, 0:512])
                        nc.sync.dma_start(aginv[2 * i + 1], vloc[:, 512:1024])

                    if fake_ag:
                        for j in range(4):
                            nc.sync.dma_start(agoutv[j], aginv)
                    else:
                        nc.gpsimd.collective_compute(
                            "AllGather",
                            mybir.AluOpType.bypass,
                            replica_groups=[[0, 1, 2, 3], [4, 5, 6, 7]],
                            ins=[aginv.opt()],
                            outs=[agoutv.opt()],
                        )

                    # ---- read back gathered v
                    vf = []
                    for i in range(16):
                        t = vfp.tile([128, C], bf16, name=f"vf{i}", tag="vf")
                        j, ii = divmod(i, 4)
                        nc.sync.dma_start(
                            t, agoutv[j, 2 * ii:2 * ii + 2, :, :].rearrange(
                                "b p e -> p b e"))
                        vf.append(t)

                    # ---- k pairs 4-7 -> AGk2
                    for p in range(4, 9):
                        if p < 8:
                            kraws[p] = rope_stage1(p, "k")
                        if p >= 5:
                            kro = rope_stage2(p - 1, "k", kraws.pop(p - 1))
                            nc.sync.dma_start(agink2[p - 5], kro)

                    if fake_ag:
                        for j in range(4):
                            nc.sync.dma_start(agoutk2[j], agink2)
                    else:
                        nc.gpsimd.collective_compute(
                            "AllGather",
                            mybir.AluOpType.bypass,
                            replica_groups=[[0, 1, 2, 3], [4, 5, 6, 7]],
                            ins=[agink2.opt()],
                            outs=[agoutk2.opt()],
                        )

                    for p in range(4, NPAIR):
                        t = kfp.tile([128, N], bf16, name=f"kf{p}", tag="kf")
                        nc.sync.dma_start(
                            t, agoutk2[:, p - 4, :, :].rearrange("r p e -> p r e"))
                        kf[p] = t

                    qro = [None] * NPAIR
                    qraws = {}
                    for p in range(NPAIR + 1):
                        if p < NPAIR:
                            qraws[p] = rope_stage1(p, "q")
                        if p >= 1:
                            qro[p - 1] = rope_stage2(p - 1, "q", qraws.pop(p - 1))

                # ---- prefetch W_proj^T (needed last; emitted after readbacks)
                wp = []
                for c in range(8):
                    t = wpp.tile([128, C], bf16, name=f"wp{c}", tag="wp")
                    nc.sync.dma_start(t, wprojT[c * 128:(c + 1) * 128, :])
                    wp.append(t)

                # ---- attention: flat slot pipeline
                recipsb = densp.tile([4, 4 * CHUNK], mybir.dt.float32r, tag="recip")
                avsb = {}
                cb_state = {}
                LAV = lag_av
                LD = lag_d
                with (
                    tc.tile_pool(name="scps", bufs=2, space="PSUM") as scpsp,
                    tc.tile_pool(name="avps", bufs=2, space="PSUM") as avpsp,
                    tc.tile_pool(name="dnps", bufs=1, space="PSUM") as dnpsp,
                    tc.tile_pool(name="normps", bufs=1, space="PSUM") as normpsp,
                    tc.tile_pool(name="pt", bufs=LAV + 2) as ptp,
                    tc.tile_pool(name="dnsb", bufs=2) as dnsbp,
                ):
                    if probe == "no_attn":
                        for pr in range(NPAIR):
                            at = attnp.tile([128, CHUNK], bf16,
                                            name=f"attn{pr}", tag="attn")
                            nc.vector.memset(at, 0.01)
                            avsb[pr] = at
                    pts = {}
                    av_t = {}
                    dn_t = {}
                    deferred = []
                    av_next = 0

                    # av lag tapers from LAV (covers AGv+readback latency)
                    # down to LAG_MIN once v has certainly landed, so the
                    # trailing av stream drains before the ACT stream ends.
                    LAG_MIN = 14
                    TAPER = 80

                    def av_target(s):
                        if s < TAPER:
                            return s - LAV
                        return min(s - LAG_MIN, 2 * s - TAPER - LAV)

                    av_emit_slot = {}
                    _nxt = 0
                    for _s in range(NSLOT + LAG_MIN):
                        while _nxt < NSLOT and _nxt <= av_target(_s):
                            av_emit_slot[_nxt] = _s
                            _nxt += 1

                    def emit_sc(s):
                        pr, g = divmod(s, KCH)
                        sc = scpsp.tile([128, 1024], f32, name=f"sc{s}", tag="sc")
                        nc.tensor.ldweights(
                            kf[pr][0:64, g * 128:(g + 1) * 128],
                            tile_position=(0, 0))
                        nc.tensor.ldweights(
                            kf[pr][64:128, g * 128:(g + 1) * 128],
                            tile_position=(64, 0))
                        nc.tensor.matmul(sc[:, 0:512],
                                         kf[pr][0:64, g * 128:(g + 1) * 128],
                                         qro[pr][0:64, :],
                                         start=True, stop=True)
                        nc.tensor.matmul(sc[:, 512:1024],
                                         kf[pr][64:128, g * 128:(g + 1) * 128],
                                         qro[pr][64:128, :],
                                         start=True, stop=True)
                        pt = ptp.tile([128, 1024], bf16, name=f"pt{s}", tag="pt")
                        nc.scalar.activation(pt, sc, Exp)
                        pts[s] = pt

                    def emit_av(s):
                        pr, g = divmod(s, KCH)
                        pt = pts[s]
                        first, last = (g == 0), (g == KCH - 1)
                        if first:
                            av_t[pr] = avpsp.tile([128, CHUNK], f32,
                                                  name=f"av{pr}", tag="av")
                        av = av_t[pr]
                        for c4 in range(4):
                            nc.tensor.ldweights(
                                vf[g][:, pr * 128 + 32 * c4:pr * 128 + 32 * (c4 + 1)],
                                tile_position=(0, 32 * c4))
                        for c4 in range(4):
                            nc.tensor.matmul(
                                av[32 * c4:32 * (c4 + 1), :],
                                vf[g][:, pr * 128 + 32 * c4:pr * 128 + 32 * (c4 + 1)],
                                pt[:, 0:512] if c4 < 2 else pt[:, 512:1024],
                                start=first, stop=last,
                                skip_group_check=True,
                                tile_position=(0, 32 * c4))
                        if last:
                            t = avsbp.tile([128, CHUNK], bf16, name=f"avsb{pr}",
                                           tag="avsb")
                            nc.vector.tensor_copy(t, av)
                            avsb[pr] = t

                    def emit_dn(s):
                        pr, g = divmod(s, KCH)
                        pt = pts[s]
                        first, last = (g == 0), (g == KCH - 1)
                        q4 = pr // 2
                        if pr % 2 == 0 and first:
                            dn_t[q4] = dnpsp.tile([128, CHUNK], f32,
                                                  name=f"dn{q4}", tag="dn")
                        dn = dn_t[q4]
                        ro = 32 * (pr % 2)
                        nc.tensor.ldweights(ones32, tile_position=(0, ro))
                        nc.tensor.ldweights(ones32, tile_position=(0, ro + 64))
                        nc.tensor.matmul(dn[ro:ro + 32, :], ones32,
                                         pt[:, 0:512], start=first, stop=last,
                                         skip_group_check=True,
                                         tile_position=(0, ro))
                        nc.tensor.matmul(dn[ro + 64:ro + 96, :], ones32,
                                         pt[:, 512:1024], start=first, stop=last,
                                         skip_group_check=True,
                                         tile_position=(0, ro + 64))
                        if pr % 2 == 1 and last:
                            # normalize chain for this quad, drip-fed
                            qs = slice(q4 * CHUNK, (q4 + 1) * CHUNK)
                            dq = dn

                            def cb_dnsb(q4=q4, dq=dq):
                                dnsb = dnsbp.tile([128, CHUNK],
                                                  mybir.dt.float32r,
                                                  name=f"dnsb{q4}", tag="dnsb")
                                with nc.allow_low_precision(reason="f32r"):
                                    nc.vector.tensor_copy(dnsb, dq)
                                cb_state[q4] = dnsb

                            def cb_dng(q4=q4):
                                dng = normpsp.tile([4, CHUNK], f32,
                                                   name=f"dng{q4}", tag="norm")
                                nc.tensor.matmul(dng, gath, cb_state[q4],
                                                 start=True, stop=True)
                                cb_state[q4] = dng

                            def cb_recip(q4=q4, qs=qs):
                                with nc.allow_low_precision(reason="f32r"):
                                    nc.vector.reciprocal(recipsb[:, qs],
                                                         cb_state[q4])

                            def mk_norm(pq, q4=q4, qs=qs):
                                def cb():
                                    rb = normpsp.tile([128, CHUNK], f32,
                                                      name=f"rb{pq}", tag="norm")
                                    nc.tensor.matmul(
                                        rb,
                                        selb[:, (pq % 2) * 128:(pq % 2 + 1) * 128],
                                        recipsb[:, qs], start=True, stop=True)
                                    at = attnp.tile([128, CHUNK], bf16,
                                                    name=f"attn{pq}", tag="attn")
                                    nc.vector.tensor_tensor(at, avsb[pq], rb,
                                                            mult)
                                    avsb[pq] = at
                                return cb

                            # ready-slot gating: norms must wait for avsb
                            s_now = 32 * q4 + 31 + LD
                            deferred.extend([
                                (s_now + 1, cb_dnsb),
                                (s_now + 2, cb_dng),
                                (s_now + 3, cb_recip),
                                (max(s_now + 4,
                                     av_emit_slot[32 * q4 + 15] + 1),
                                 mk_norm(2 * q4)),
                                (av_emit_slot[32 * q4 + 31] + 1,
                                 mk_norm(2 * q4 + 1)),
                            ])

                    if probe != "no_attn":
                        for s in range(NSLOT + LAG_MIN):
                            if s < NSLOT:
                                emit_sc(s)
                            if s >= LD and s - LD < NSLOT:
                                emit_dn(s - LD)
                            # tapered av lag: emit every av whose slot is due
                            while (av_next < NSLOT
                                   and av_next <= av_target(s)):
                                emit_av(av_next)
                                pts.pop(av_next, None)
                                av_next += 1
                            # drip deferred callbacks whose deps are ready
                            while deferred and deferred[0][0] <= s:
                                deferred.pop(0)[1]()
                        while deferred:
                            deferred.pop(0)[1]()

                attnT = [avsb[p] for p in range(NPAIR)]

                # ---- output projection + bias
                with (
                    tc.tile_pool(name="yps", bufs=4, space="PSUM") as ypsp,
                    tc.tile_pool(name="ysb", bufs=2) as ysbp,
                ):
                    for oc in range(8):
                        yp = ypsp.tile([128, CHUNK], f32, name=f"yp{oc}", tag="yp")
                        for c in range(8):
                            nc.tensor.matmul(yp, wp[c][:, oc * 128:(oc + 1) * 128],
                                             attnT[c], start=(c == 0), stop=(c == 7))
                        ysb = ysbp.tile([128, CHUNK], f32, name=f"ysb{oc}", tag="ysb")
                        nc.vector.tensor_scalar(ysb, yp, bias[:, oc:oc + 1], None, add)
                        nc.sync.dma_start(out_p[oc * 128:(oc + 1) * 128, :], ysb)

    nc.finalize()
    return nc


# ------------------------------------------------------------------- kernel

def prepare_in_maps(x, W_qkv, W_proj, b_proj, T, H, W):
    T, H, W_ = int(T), int(H), int(W)
    x = np.asarray(x, np.float32)
    W_qkv = np.asarray(W_qkv, np.float32)
    W_proj = np.asarray(W_proj, np.float32)
    b_proj = np.asarray(b_proj, np.float32)
    assert x.shape == (B, N, C) and T * H * W_ == N

    scale = HD ** -0.5
    cos_full, sin_full = _build_tables(T, H, W_)
    wqkvT = np.ascontiguousarray(W_qkv.T).astype(BF16)
    wprojT = np.ascontiguousarray(W_proj.T).astype(BF16)
    smat = _build_S128().astype(BF16)
    bproj2 = b_proj.reshape(C, 1).astype(np.float32)
    # dng rows after gath: {0: even-pair h0, 1: odd-pair h0,
    #                       2: even-pair h1, 3: odd-pair h1}
    # selb[:, 0:128]: even pair of a quad; [:, 128:256]: odd pair
    selb = np.zeros((4, 256), np.float32)
    selb[0, 0:64] = 1.0
    selb[2, 64:128] = 1.0
    selb[1, 128:192] = 1.0
    selb[3, 192:256] = 1.0
    # dn quad-tile rows {0, 32, 64, 96} = {e-h0, o-h0, e-h1, o-h1}
    gath = np.zeros((128, 4), np.float32)
    for r in range(4):
        gath[32 * r, r] = 1.0

    in_maps = []
    for core in range(NCORES):
        b, j = divmod(core, 4)
        r0 = j * CHUNK
        sl = slice(r0, r0 + CHUNK)
        cos_l = cos_full[sl].T
        sin_l = sin_full[sl].T
        cq = np.concatenate([cos_l, cos_l], 0) * scale
        sq = np.concatenate([sin_l, sin_l], 0) * scale
        ck = np.concatenate([cos_l, cos_l], 0)
        sk = np.concatenate([sin_l, sin_l], 0)
        in_maps.append({
            "xT": np.ascontiguousarray(x[b, sl, :].T).astype(BF16),
            "wqkvT": wqkvT,
            "wprojT": wprojT,
            "bproj": bproj2,
            "cosq": np.ascontiguousarray(cq, np.float32),
            "sinq": np.ascontiguousarray(sq, np.float32),
            "cosk": np.ascontiguousarray(ck, np.float32),
            "sink": np.ascontiguousarray(sk, np.float32),
            "smat": smat,
            "selb": selb,
            "gath": gath,
        })
    return in_maps


def assemble_output(results):
    y = np.empty((B, N, C), np.float32)
    for core in range(NCORES):
        b, j = divmod(core, 4)
        r0 = j * CHUNK
        y[b, r0:r0 + CHUNK, :] = results[core]["out"].T
    return y


def get_nc(reps=1, fake_ag=False, probe=None, lag_av=36, lag_d=2):
    key = ("nc", reps, fake_ag, probe, lag_av, lag_d)
    if key not in _CACHE:
        _CACHE[key] = _build_nc(reps, fake_ag, probe, lag_av, lag_d)
    return _CACHE[key]


def kernel(x, W_qkv, W_proj, b_proj, T, H, W):
    from concourse.bass_utils import run_bass_kernel_spmd

    nc = get_nc()
    in_maps = prepare_in_maps(x, W_qkv, W_proj, b_proj, T, H, W)
    res = run_bass_kernel_spmd(nc, in_maps, core_ids=list(range(NCORES)))
    return assemble_output(res.results)


if __name__ == "__main__":
    rng = np.random.default_rng(0)
    inp = {
        "x": rng.standard_normal((B, N, C), np.float32),
        "W_qkv": rng.standard_normal((3 * C, C), np.float32) * 0.02,
        "W_proj": rng.standard_normal((C, C), np.float32) * 0.02,
        "b_proj": rng.standard_normal(C, np.float32) * 0.02,
        "T": 8, "H": 16, "W": 16,
    }
    out = kernel(**inp)
    print(out.shape, out.dtype)
